# revision 29
# baseline (speedup 1.0000x reference)
"""Trainium2 Bass kernel for DecomposedQValueNN (gnn_message_passing).

Per batch row b of x[65536, 128]:
  xa = x.reshape(B, 32, 4); other_a = MLP_o(xa[:,a]) (3 relu layers, 4-32-32-16)
  sum_other = sum_{a != sel} other_a;  sel_out = MLP_s(xa[:,sel])
  h = relu([sel_out; sum_other] @ gW1 + gb1); q = h @ gW2 + gb2
  out[b] = q[b, clip(int(xa[b,sel,3]),0,1)]

V2 design (8 cores, batch data-parallel, 8192 rows/core):
  - host transposes + bf16-casts x to [feat=128, rows] so no on-device
    transpose is needed and DMA reads are contiguous per partition
  - all matmuls bf16 (single-pass on PE vs fp32's 2-pass); PSUM fp32
  - 32x32 PE-array tiling, loop order cycles row groups so LDWEIGHTS
    overlaps in-flight matmuls
  - PSUM pair-tiles [128,1024] (2 banks) so each PSUM->SBUF evacuation
    instruction covers 2 banks; evacs alternate ScalarE/VectorE
  - K=64 two-agent packing in L3 so z3 fills PSUM banks completely
  - global head folded into the agent-sum: accumulating matmuls against
    replicated gW1_sum (sel agent excluded via a zeroed weight variant),
    plus one K=16 matmul adding gW1_sel^T @ sel_out
  - sel-MLP batched across 4 chunks via diagonal tile-position matmuls
  - software pipelining: each chunk's global-head tail is emitted 4 chunks
    later, interleaved between the following chunks' layer blocks; drain
    tails at the end run pairwise-interleaved
  Final 2-way q gather on host.
"""

import numpy as np
import ml_dtypes

BF16 = ml_dtypes.bfloat16

B_FULL = 65536
N_CORES = 8
B_C = B_FULL // N_CORES       # 8192
A, D = 32, 4
NCH = 512                     # batch cols per PSUM bank (fp32)
CHUNKS = B_C // NCH           # 16

# wpack (bf16) column offsets
OW1BD = 0          # [128, 256] L1 block-diag variants u=0..7
OW2 = 256          # [128, 32]
OW3P = 288         # [128, 32] K=64 2-agent pack: even row-grp W3->cols 0..15,
                   # odd row-grp W3->cols 16..31
OSW1 = 320         # [128, 32]
OSW2 = 352
OSW3 = 384
OGSUMF = 416       # [128, 32] gW1[16+(m%16)] at every partition 32j+m
OGSUME = 448       # same, sel agent's 16-row stripe zeroed
OGSEL = 480        # [16, 32] gW1[0:16]
OGW2 = 512         # [32, 32] gW2 padded
WCOLS = 544

_COMPILED = {}
LAST_RESULT = None


def _f32(a):
    return np.ascontiguousarray(a, dtype=np.float32)


def _build_wpack(sel, oW1, oW2, oW3, sW1, sW2, sW3, gW1, gW2):
    P = 128

    def rep4(w):              # [32, m] -> [128, m]
        return np.tile(_f32(w), (4, 1))

    def padc(w, m):
        w = _f32(w)
        return np.concatenate([w, np.zeros((w.shape[0], m - w.shape[1]), np.float32)], 1)

    def padr(w, m):
        w = _f32(w)
        return np.concatenate([w, np.zeros((m - w.shape[0], w.shape[1]), np.float32)], 0)

    # L1 weights as zero-padded K=32 blocks: block u has oW1 at rows 4u..4u+3
    w1bd = np.zeros((32, 8 * 32), np.float32)
    for u in range(8):
        w1bd[4 * u:4 * u + 4, 32 * u:32 * u + 32] = _f32(oW1)
    w1 = np.tile(w1bd, (4, 1))
    w2 = rep4(oW2)
    # K=64 2-agent pack: row-group r even -> z3 cols 0..15, odd -> 16..31
    w3L = padc(_f32(oW3), 32)                       # [32, 32] cols 0..15
    w3R = np.concatenate([np.zeros((32, 16), np.float32), _f32(oW3)], 1)
    w3 = np.concatenate([w3L, w3R, w3L, w3R], 0)    # [128, 32]
    u_sel = sel % 8
    sw1p = np.zeros((32, 32), np.float32)
    sw1p[4 * u_sel:4 * u_sel + 4, :] = _f32(sW1)
    sw1 = np.tile(sw1p, (4, 1))
    sw2 = rep4(_f32(sW2))
    sw3 = rep4(padc(sW3, 32))

    # z3s layout (K=64 packed): per p one pair-tile; half k=i1//2 holds
    # agents (8*i1+4p+j) at partitions 32j+0..15 and (8*(i1+1)+4p+j) at
    # 32j+16..31.  gsum weight: row 32j+m -> gW1[16+(m%16), :].
    g = _f32(gW1)
    gsumf = np.zeros((P, 32), np.float32)
    for j in range(4):
        gsumf[32 * j:32 * j + 16, :] = g[16:32, :]
        gsumf[32 * j + 16:32 * j + 32, :] = g[16:32, :]
    gsume = gsumf.copy()
    j_sel = sel % 4
    i_sel = sel // 8
    off = 32 * j_sel + 16 * (i_sel % 2)
    gsume[off:off + 16, :] = 0.0

    # gsel replicated: rows 32c+m (m<16) = gW1[m] (batched sel tile stripes)
    gselw = np.zeros((P, 32), np.float32)
    for c in range(4):
        gselw[32 * c:32 * c + 16, :] = g[0:16, :]
    gw2 = padr(padc(gW2, 32), P)         # wp[0:32, OGW2:]

    parts = [w1, w2, w3, sw1, sw2, sw3, gsumf, gsume, gselw, gw2]
    wp = np.concatenate(parts, axis=1)
    assert wp.shape == (P, WCOLS), wp.shape
    return np.ascontiguousarray(wp.astype(BF16))


def _build_bias(ob1, ob2, ob3, sb1, sb2, sb3, gb1, gb2):
    # fp32 per-partition bias columns: col k pattern = b[(p % 32) % len]
    P = 128

    def bias_col(b, valid=32):
        v = np.zeros((P, 1), np.float32)
        b = _f32(b).ravel()
        for p in range(P):
            r = p % 32
            if r < valid:
                v[p, 0] = b[r % len(b)]
        return v

    cols = [bias_col(ob1), bias_col(ob2), bias_col(ob3, 32),
            bias_col(sb1), bias_col(sb2), bias_col(sb3, 16),
            bias_col(gb1), bias_col(gb2, 2)]
    return np.ascontiguousarray(np.concatenate(cols, 1))  # [128, 8] fp32


BB1, BB2, BB3, BSB1, BSB2, BSB3, BGB1, BGB2 = range(8)


def _build_nc(sel):
    import concourse.bacc as bacc
    import concourse.mybir as mybir
    from concourse.tile import TileContext

    f32 = mybir.dt.float32
    bf16 = mybir.dt.bfloat16
    Relu = mybir.ActivationFunctionType.Relu
    Ident = mybir.ActivationFunctionType.Identity
    add_op = mybir.AluOpType.add
    max_op = mybir.AluOpType.max

    i_sel, j_sel = sel // 8, sel % 4
    p_sel = (sel % 8) // 4
    si = sel // 8          # partition group of sel agent's features

    nc = bacc.Bacc("TRN2", target_bir_lowering=False, debug=False,
                   num_devices=N_CORES)
    x_ext = nc.dram_tensor("xt", [128, B_C], bf16, kind="ExternalInput").ap()
    w_ext = nc.dram_tensor("wpack", [128, WCOLS], bf16, kind="ExternalInput").ap()
    b_ext = nc.dram_tensor("bias", [128, 8], f32, kind="ExternalInput").ap()
    o_ext = nc.dram_tensor("out", [2, B_C], f32, kind="ExternalOutput").ap()

    with TileContext(nc) as tc:
        with (
            tc.tile_pool(name="const", bufs=1) as cpool,
            tc.tile_pool(name="xin", bufs=8) as xpool,
            tc.tile_pool(name="h", bufs=8) as hpool,
            tc.tile_pool(name="z3s", bufs=11) as z3pool,
            tc.tile_pool(name="gl", bufs=3) as glpool,
            tc.tile_pool(name="osb", bufs=1) as opool,
            tc.tile_pool(name="zp", bufs=3, space="PSUM") as zpool,
            tc.tile_pool(name="gp", bufs=2, space="PSUM") as gpool,
        ):
            W = cpool.tile([128, WCOLS], bf16, name="W")
            nc.sync.dma_start(out=W[:], in_=w_ext[:])
            BI = cpool.tile([128, 8], f32, name="BI")
            nc.sync.dma_start(out=BI[:], in_=b_ext[:])
            outsb = opool.tile([2, B_C], f32, name="outsb")

            def evac(dst, src, bcol, func, dve, lo=0, size=128):
                b = BI[lo:lo + size, bcol:bcol + 1]
                if dve:
                    if func == "relu":
                        nc.vector.tensor_scalar(dst, src, b, 0.0, add_op, max_op)
                    else:
                        nc.vector.tensor_scalar_add(dst, src, b)
                else:
                    if func == "relu":
                        nc.scalar.activation(dst, src, Relu, bias=b)
                    else:
                        nc.scalar.activation(dst, src, Ident, bias=b)

            xin_t = {}
            z3s_all = {}
            bsh3_g = {}

            def emit_L1_half(c, p):
                if p == 0:
                    b0 = c * NCH
                    xin = xpool.tile([128, NCH], bf16, tag="xin", name=f"xin{c}")
                    nc.sync.dma_start(out=xin[:], in_=x_ext[:, b0:b0 + NCH])
                    xin_t[c] = xin
                xin = xin_t[c]
                z1 = [zpool.tile([128, 2 * NCH], f32, tag="z",
                                 name=f"z1_{c}_{p}_{k}") for k in range(2)]
                for j in range(4):
                    for i in range(4):
                        u = 4 * p + j
                        nc.tensor.matmul(
                            z1[i // 2][32 * j:32 * j + 32,
                                       NCH * (i % 2):NCH * (i % 2) + NCH],
                            W[32 * i:32 * i + 32,
                              OW1BD + 32 * u:OW1BD + 32 * u + 32],
                            xin[32 * i:32 * i + 32, :],
                            start=True, stop=True,
                            tile_position=(32 * i, 32 * j))
                out = {}
                for k in range(2):
                    h1 = hpool.tile([128, 2 * NCH], bf16, tag="h1",
                                    name=f"h1_{c}_{p}_{k}")
                    evac(h1[:], z1[k][:], BB1, "relu", dve=(k == 0))
                    out[k] = h1
                return out

            def emit_L2_half(c, p, h1s):
                z2 = [zpool.tile([128, 2 * NCH], f32, tag="z",
                                 name=f"z2_{c}_{p}_{k}") for k in range(2)]
                for i in range(4):
                    for j in range(4):
                        nc.tensor.matmul(
                            z2[j // 2][32 * i:32 * i + 32,
                                       NCH * (j % 2):NCH * (j % 2) + NCH],
                            W[32 * j:32 * j + 32, OW2:OW2 + 32],
                            h1s[i // 2][32 * j:32 * j + 32,
                                        NCH * (i % 2):NCH * (i % 2) + NCH],
                            start=True, stop=True,
                            tile_position=(32 * j, 32 * i))
                out = {}
                for k in range(2):
                    h2 = hpool.tile([128, 2 * NCH], bf16, tag="h2",
                                    name=f"h2_{c}_{p}_{k}")
                    evac(h2[:], z2[k][:], BB2, "relu", dve=(k == 0))
                    out[k] = h2
                return out

            def emit_L3_half(c, p, h2s):
                z3 = zpool.tile([128, 2 * NCH], f32, tag="z", name=f"z3_{c}_{p}")
                for j in range(4):
                    for i1 in (0, 2):
                        nc.tensor.matmul(
                            z3[32 * j:32 * j + 32,
                               NCH * (i1 // 2):NCH * (i1 // 2) + NCH],
                            W[32 * i1:32 * i1 + 64, OW3P:OW3P + 32],
                            h2s[j // 2][32 * i1:32 * i1 + 64,
                                        NCH * (j % 2):NCH * (j % 2) + NCH],
                            start=True, stop=True,
                            tile_position=(32 * i1, 32 * j))
                z3sb = z3pool.tile([128, 2 * NCH], bf16, tag="z3s",
                                   name=f"z3s_{c}_{p}")
                evac(z3sb[:], z3[:], BB3, "relu", dve=(p == 0 and c % 2 == 0))
                z3s_all.setdefault(c, {})[p] = z3sb

            def emit_selb(grp):
                # batched sel-MLP for chunks 4g..4g+3: chunk stripe = 32*(c%4)
                bz1 = gpool.tile([128, NCH], f32, tag="g", name=f"bz1_{grp}")
                for cl in range(4):
                    cc = 4 * grp + cl
                    nc.tensor.matmul(
                        bz1[32 * cl:32 * cl + 32, :],
                        W[32 * si:32 * si + 32, OSW1:OSW1 + 32],
                        xin_t[cc][32 * si:32 * si + 32, :],
                        start=True, stop=True,
                        tile_position=(32 * si, 32 * cl))
                bsh1 = glpool.tile([128, NCH], bf16, tag="sh1", name=f"bsh1_{grp}")
                evac(bsh1[:], bz1[:], BSB1, "relu", dve=True)
                bz2 = gpool.tile([128, NCH], f32, tag="g", name=f"bz2_{grp}")
                for cl in range(4):
                    nc.tensor.matmul(
                        bz2[32 * cl:32 * cl + 32, :],
                        W[32 * cl:32 * cl + 32, OSW2:OSW2 + 32],
                        bsh1[32 * cl:32 * cl + 32, :],
                        start=True, stop=True,
                        tile_position=(32 * cl, 32 * cl))
                bsh2 = glpool.tile([128, NCH], bf16, tag="sh2", name=f"bsh2_{grp}")
                evac(bsh2[:], bz2[:], BSB2, "relu", dve=False)
                bz3 = gpool.tile([128, NCH], f32, tag="g", name=f"bz3_{grp}")
                for cl in range(4):
                    nc.tensor.matmul(
                        bz3[32 * cl:32 * cl + 32, :],
                        W[32 * cl:32 * cl + 32, OSW3:OSW3 + 32],
                        bsh2[32 * cl:32 * cl + 32, :],
                        start=True, stop=True,
                        tile_position=(32 * cl, 32 * cl))
                bsh3 = glpool.tile([128, NCH], bf16, tag="sh3", name=f"bsh3_{grp}")
                evac(bsh3[:], bz3[:], BSB3, "relu", dve=True)
                bsh3_g[grp] = bsh3

            GSUM_ORDER = [(0, 0), (0, 1), (1, 0), (1, 1)]

            def emit_gsum_mm(c, zg, step):
                p, k = GSUM_ORDER[step]
                excl = (p == p_sel and k == i_sel // 2)
                oo = OGSUME if excl else OGSUMF
                nc.tensor.matmul(
                    zg[:], W[:, oo:oo + 32],
                    z3s_all[c][p][:, NCH * k:NCH * k + NCH],
                    start=(step == 0), stop=False)

            def emit_gsel_mm(c, zg):
                cl = c % 4
                bsh3 = bsh3_g[c // 4]
                nc.tensor.matmul(
                    zg[:], W[32 * cl:32 * cl + 16, OGSEL:OGSEL + 32],
                    bsh3[32 * cl:32 * cl + 16, :],
                    start=False, stop=True, tile_position=(32 * cl, 0))

            def emit_tail_fin(c, zg):
                b0 = c * NCH
                hg = glpool.tile([32, NCH], bf16, tag="hg", name=f"hg_{c}")
                evac(hg[:], zg[:], BGB1, "relu", dve=False, size=32)
                qp = gpool.tile([32, NCH], f32, tag="g", name=f"qp_{c}")
                nc.tensor.matmul(
                    qp[:], W[0:32, OGW2:OGW2 + 32], hg[0:32, :],
                    start=True, stop=True, tile_position=(0, 0))
                evac(outsb[0:2, b0:b0 + NCH], qp[0:2, :], BGB2, "add",
                     dve=False, size=2)
                del z3s_all[c]

            def emit_tail_all(c):
                zg = gpool.tile([32, NCH], f32, tag="g", name=f"zg_{c}")
                for st in range(4):
                    emit_gsum_mm(c, zg, st)
                emit_gsel_mm(c, zg)
                emit_tail_fin(c, zg)

            LAG = 4  # tail(c) needs bsh3 of group c//4 (ready in chunk 4g+3)
            for c in range(CHUNKS):
                tail = c - LAG if c >= LAG else None
                h1s_p0 = emit_L1_half(c, 0)
                h1s_p1 = emit_L1_half(c, 1)
                if tail is not None:
                    zg = gpool.tile([32, NCH], f32, tag="g", name=f"zg_{tail}")
                    for st in range(3):
                        emit_gsum_mm(tail, zg, st)
                h2s_p0 = emit_L2_half(c, 0, h1s_p0)
                h2s_p1 = emit_L2_half(c, 1, h1s_p1)
                if tail is not None:
                    emit_gsum_mm(tail, zg, 3)
                    emit_gsel_mm(tail, zg)
                emit_L3_half(c, 0, h2s_p0)
                emit_L3_half(c, 1, h2s_p1)
                if tail is not None:
                    emit_tail_fin(tail, zg)
                if c % 4 == 3:
                    emit_selb(c // 4)
            # drain: remaining LAG tails, pairwise-interleaved so the
            # accumulation chains of two tails overlap on the PE
            for t0 in range(CHUNKS - LAG, CHUNKS, 2):
                zgs = {}
                for t in (t0, t0 + 1):
                    zgs[t] = gpool.tile([32, NCH], f32, tag="g", name=f"zg_{t}")
                for st in range(4):
                    for t in (t0, t0 + 1):
                        emit_gsum_mm(t, zgs[t], st)
                for t in (t0, t0 + 1):
                    emit_gsel_mm(t, zgs[t])
                for t in (t0, t0 + 1):
                    emit_tail_fin(t, zgs[t])

            nc.sync.dma_start(out=o_ext[:], in_=outsb[:])
    nc.compile()
    return nc


def kernel(**inputs):
    x = _f32(inputs["joint_state_actions"])
    sel = int(inputs["selected_agent_idx"])

    wpack = _build_wpack(
        sel, inputs["oW1"], inputs["oW2"], inputs["oW3"],
        inputs["sW1"], inputs["sW2"], inputs["sW3"],
        inputs["gW1"], inputs["gW2"])
    bias = _build_bias(
        inputs["ob1"], inputs["ob2"], inputs["ob3"],
        inputs["sb1"], inputs["sb2"], inputs["sb3"],
        inputs["gb1"], inputs["gb2"])

    if sel not in _COMPILED:
        _COMPILED[sel] = _build_nc(sel)
    nc = _COMPILED[sel]

    from concourse.bass_utils import run_bass_kernel_spmd
    shards = [np.ascontiguousarray(x[i * B_C:(i + 1) * B_C].T.astype(BF16))
              for i in range(N_CORES)]
    in_maps = [{"xt": s, "wpack": wpack, "bias": bias} for s in shards]
    import os
    trace = bool(int(os.environ.get("KERNEL_TRACE", "0")))
    res = run_bass_kernel_spmd(nc, in_maps, list(range(N_CORES)),
                               trace=trace)
    global LAST_RESULT
    LAST_RESULT = res

    q01 = np.concatenate([res.results[i]["out"] for i in range(N_CORES)],
                         axis=1)
    act = np.clip(x[:, 4 * sel + 3].astype(np.int32), 0, 1)
    out = np.where(act == 0, q01[0], q01[1]).astype(np.float32)
    return out[:, None]
